# revision 10
# baseline (speedup 1.0000x reference)
"""Causal self-attention (query-axis softmax) for Trainium2, 8 NeuronCores.

Sharding: 8 cores = 4 batches x 2 half-head-groups. Core c handles batch
c//2 and heads (c%2)*6 .. (c%2)*6+5. Each core computes its heads' full
attention plus its partial output projection; the host sums the two
partials per batch and adds b_proj.

Layout strategy per core (T=2048, C=768, 6 heads, hd=64):
  - host passes x[b].T so the QKV contraction dim (C) lands on SBUF
    partitions without any on-chip transpose.
  - fp8 fast path: x and the weights are cast to fp8e4 on the host
    (weights pre-scaled by 64 so w*64 ~ N(0,1.3) sits in fp8's sweet
    range). QKV and the output projection run as fp8 DoubleRow matmuls
    (two 128-row contraction subtiles per instruction, 2x bf16 FLOP
    rate). The 64x weight scale flows through: q,k carry 64x (folded
    into the exp scale as 1/4096), v carries 64x (folds into the fp8
    value range for AV and a 1/4096 descale on the proj output).
  - Q,K are produced transposed ([head_d, t]) so S^T = K Q^T tiles have
    softmax's query axis on the free dimension. S^T runs in bf16 (its
    contraction is only 64, so DoubleRow can't help); the 64-wide
    operands are zero-padded to full 128x128 mode because tiled matmul
    modes run at the cold 1.2 GHz clock.
  - AV runs as fp8 DoubleRow over kt-tile PAIRS: at (the exp output) is
    written by the scalar engine directly in fp8 as [128, 2, T] pair
    tiles, V is scaled by 1/denom into fp8 pair tiles. The two heads of
    a pair share one y^T psum tile via zero-padded stationaries. AV for
    pair p is emitted after pair p+1's S^T so the in-order PE stream
    never blocks on the softmax chain.
  - softmax over q (free axis): no max-subtraction needed (logits are
    O(1) by construction), exp+rowsum fused on ScalarE via accum_out
    over 1024-wide PSUM chunks; normalization folded into V rows.
  - causal mask: ragged chunk bounds skip fully-masked blocks; diagonal
    128x128 blocks get a precomputed triangular -1e6 add (on the raw
    4096x-scaled logits). The odd kt tile of each pair has a 128-col
    never-exp'd block in its at pair-tile; gpsimd zero-fills it.
"""

import os
import sys

sys.path.insert(0, "/opt/trn_rl_repo")

import numpy as np
import ml_dtypes

import concourse.bass as bass
import concourse.mybir as mybir
import concourse.tile as tile
from concourse.bass_utils import run_bass_kernel_spmd

FP32 = mybir.dt.float32
BF16 = mybir.dt.bfloat16
FP8 = mybir.dt.float8e4
U8 = mybir.dt.uint8
DR = mybir.MatmulPerfMode.DoubleRow
NPF8 = ml_dtypes.float8_e4m3fn

B, T, C, H = 4, 2048, 768, 12
D = 64                  # head dim
NCORES = 8
HPC = H * B // NCORES   # heads per core = 6
E = HPC * D             # qkv slice width per core = 384
CT = C // 128           # c tiles = 6
CTP = CT // 2           # ct pairs = 3
ET = E // 128           # e tiles = 3
TT = T // 128           # t tiles = 16
NKP = TT // 2           # kt pairs = 8
PBF = 6                 # kt pairs >= PBF run bf16 AV (concentrated
                        # softmax weights there; fp8's 3-bit mantissa
                        # error would not average out over few terms)
QCH = 512               # matmul moving chunk (PSUM bank limit)
NQC = T // QCH          # 4
BCH = 1024              # exp chunk
NBC = T // BCH          # 2
WS = 64.0               # host-side weight scale
MASKV = -1.0e6          # on raw (4096x) logits: *SCALE2 = -30.5
SCALE2 = 0.125 / (WS * WS)   # exp scale: 1/sqrt(hd) / (64*64)
ODESC = 1.0 / WS             # proj psum descale (y carries 64x, wp unscaled)
Exp = mybir.ActivationFunctionType.Exp


def _split_sync_waits(nc):
    """This container's walrus encodes at most one sync wait per
    instruction for several instruction structs; hoist extra waits onto
    same-engine nops placed immediately before the instruction."""
    for f in nc.m.functions:
        for bb in f.blocks:
            new_insts = []
            for inst in bb.instructions:
                si = inst.sync_info
                waits = list(si.on_wait) if si is not None and si.on_wait else []
                if len(waits) > 1:
                    for w in waits[:-1]:
                        nop = mybir.InstNoOp(
                            name=nc.get_next_instruction_name(),
                            engine=inst.engine,
                            sync_info=mybir.SyncInfo(on_wait=[w], on_update=[]),
                            bass_nofuse=True,
                        )
                        nc.register_instruction(nop)
                        new_insts.append(nop)
                    inst.sync_info = mybir.SyncInfo(
                        on_wait=[waits[-1]], on_update=list(si.on_update or [])
                    )
                new_insts.append(inst)
            bb.instructions[:] = new_insts


def _build():
    nc = bass.Bass("TRN2")
    xT = nc.dram_tensor("xT", [NQC, 128, CT, QCH], FP8, kind="ExternalInput")
    xB = nc.dram_tensor("xB", [NQC, 128, CT, QCH], BF16, kind="ExternalInput")
    wq = nc.dram_tensor("wq", [128, CT, E], FP8, kind="ExternalInput")
    wk = nc.dram_tensor("wk", [128, CT, E], FP8, kind="ExternalInput")
    wv = nc.dram_tensor("wv", [128, CT, E], BF16, kind="ExternalInput")
    bq = nc.dram_tensor("bq", [E], FP32, kind="ExternalInput")
    bk = nc.dram_tensor("bk", [E], FP32, kind="ExternalInput")
    bv = nc.dram_tensor("bv", [E], FP32, kind="ExternalInput")
    wp = nc.dram_tensor("wp", [128, ET, C], BF16, kind="ExternalInput")
    mask = nc.dram_tensor("mask", [128, 128], FP32, kind="ExternalInput")
    out = nc.dram_tensor("out", [T, C], FP32, kind="ExternalOutput")

    with tile.TileContext(nc) as tc:
        with (
            tc.tile_pool(name="wts", bufs=1) as wts,
            tc.tile_pool(name="xp", bufs=2) as xp,
            tc.tile_pool(name="big", bufs=1) as big,
            tc.tile_pool(name="atp", bufs=4) as atp,
            tc.tile_pool(name="sm", bufs=4) as sm,
            tc.tile_pool(name="op", bufs=3) as op,
        ):
            # ---- constant loads ----
            xt_pre = []
            for tci in range(2):
                xt = xp.tile([128, CT, QCH], FP8, tag="xt", bufs=2, name="xt")
                nc.sync.dma_start(out=xt, in_=xT[tci])
                xb = xp.tile([128, CT, QCH], BF16, tag="xb", bufs=2, name="xb")
                nc.sync.dma_start(out=xb, in_=xB[tci])
                xt_pre.append((xt, xb))
            wq_sb = wts.tile([128, CT, E], FP8)
            wk_sb = wts.tile([128, CT, E], FP8)
            wv_sb = wts.tile([128, CT, E], BF16)
            for ct in range(CT):
                nc.sync.dma_start(out=wq_sb[:, ct, :], in_=wq[:, ct, :])
            for ct in range(CT):
                nc.sync.dma_start(out=wk_sb[:, ct, :], in_=wk[:, ct, :])
            for ct in range(CT):
                nc.sync.dma_start(out=wv_sb[:, ct, :], in_=wv[:, ct, :])
            bq_sb = wts.tile([128, ET], FP32)
            bk_sb = wts.tile([128, ET], FP32)
            nc.sync.dma_start(out=bq_sb, in_=bq.rearrange("(et p) -> p et", p=128))
            nc.sync.dma_start(out=bk_sb, in_=bk.rearrange("(et p) -> p et", p=128))
            bv_sb = wts.tile([128, E], FP32)
            nc.sync.dma_start(out=bv_sb, in_=bv[None, :].to_broadcast((128, E)))
            mask_sb = wts.tile([128, 128], FP32)
            nc.sync.dma_start(out=mask_sb, in_=mask[:])
            wp_sb = wts.tile([128, ET, C], BF16)
            nc.sync.dma_start(out=wp_sb, in_=wp[:])

            qt2 = big.tile([128, ET, T], BF16)      # [d-in-pair, pair, t] (64x)
            ktp2 = big.tile([128, ET, 2, T], BF16)  # [d(+zero half), pair, head-in-pair, t]
            v_sb = big.tile([128, TT, E], FP32)     # [t-in-tile, ttile, (head,d)] (64x)
            yb = big.tile([128, ET, T], BF16)       # [hd-in-pair, pair, t] (64x)
            nc.gpsimd.memset(ktp2.bitcast(mybir.dt.uint16), 0)
            # persistent rotating fp8 Vs pair tiles, 2 per head-in-pair slot;
            # head A tiles keep cols 64:128 zero, head B tiles cols 0:64 zero.
            vspad = [
                [big.tile([128, 2, 128], FP8, name=f"vspad{j}_{i}") for i in range(2)]
                for j in range(2)
            ]
            vspad_bf = [
                [big.tile([128, 128], BF16, name=f"vspbf{j}_{i}") for i in range(2)]
                for j in range(2)
            ]
            for row in vspad:
                for t_ in row:
                    nc.gpsimd.memset(t_.bitcast(U8), 0)
            for row in vspad_bf:
                for t_ in row:
                    nc.gpsimd.memset(t_.bitcast(mybir.dt.uint16), 0)

            # ---- phase 0: V and Q/K for all heads (fp8 DoubleRow) ----
            with tc.tile_pool(name="psA", bufs=4, space="PSUM") as psA:
                for tci in range(NQC):
                    if tci < 2:
                        xt, xb = xt_pre[tci]
                    else:
                        xt = xp.tile([128, CT, QCH], FP8, tag="xt", bufs=2, name="xt")
                        nc.sync.dma_start(out=xt, in_=xT[tci])
                        xb = xp.tile([128, CT, QCH], BF16, tag="xb", bufs=2, name="xb")
                        nc.sync.dma_start(out=xb, in_=xB[tci])
                    cols = slice(tci * QCH, (tci + 1) * QCH)
                    for et in range(ET):
                        pq = psA.tile([128, QCH], FP32, tag="ps", bufs=4, name="pq")
                        for cp in range(CTP):
                            nc.tensor.matmul(
                                pq,
                                wq_sb[:, 2 * cp:2 * cp + 2, et * 128:(et + 1) * 128],
                                xt[:, 2 * cp:2 * cp + 2, :],
                                start=(cp == 0), stop=(cp == CTP - 1),
                                perf_mode=DR,
                            )
                        nc.vector.tensor_scalar_add(qt2[:, et, cols], pq, bq_sb[:, et:et + 1])
                        pk = psA.tile([128, QCH], FP32, tag="ps", bufs=4, name="pk")
                        for cp in range(CTP):
                            nc.tensor.matmul(
                                pk,
                                wk_sb[:, 2 * cp:2 * cp + 2, et * 128:(et + 1) * 128],
                                xt[:, 2 * cp:2 * cp + 2, :],
                                start=(cp == 0), stop=(cp == CTP - 1),
                                perf_mode=DR,
                            )
                        nc.vector.tensor_scalar_add(
                            ktp2[0:64, et, 0, cols], pk[0:64, :], bk_sb[0:64, et:et + 1]
                        )
                        nc.vector.tensor_scalar_add(
                            ktp2[64:128, et, 1, cols], pk[64:128, :], bk_sb[64:128, et:et + 1]
                        )
                    for ttl in range(4):
                        tt = tci * 4 + ttl
                        pv = psA.tile([128, QCH], FP32, tag="ps", bufs=4, name="pv")
                        for ct in range(CT):
                            nc.tensor.matmul(
                                pv[:, :E],
                                xb[:, ct, ttl * 128:(ttl + 1) * 128],
                                wv_sb[:, ct, :],
                                start=(ct == 0), stop=(ct == CT - 1),
                            )
                        nc.vector.tensor_add(v_sb[:, tt, :], pv[:, :E], bv_sb)

            # ---- attention ----
            # Per head pair hp, kt tiles are processed in PAIRS (DoubleRow).
            # The two heads of a pair interleave and SHARE one y^T psum tile
            # via zero-padded Vs stationaries (accumulating zero is a no-op).
            with (
                tc.tile_pool(name="psS", bufs=2, space="PSUM") as psS,
                tc.tile_pool(name="psY", bufs=1, space="PSUM") as psY,
            ):
                for hp in range(ET):
                    yps = psY.tile([128, T], FP32, tag="y", name="yps")
                    pend = []  # [(kind, hj, p-or-kt, at, vsp)]

                    def emit_av(kind0, hj0, i0, at0, vsp0):
                        if kind0 == "dr":
                            klo0 = 256 * i0
                            for qc in range(i0 // 2, NQC):
                                lo = max(QCH * qc, klo0)
                                hi = QCH * qc + QCH
                                nc.tensor.matmul(
                                    yps[:, lo:hi], vsp0, at0[:, :, lo:hi],
                                    start=(i0 == 0 and hj0 == 0),
                                    stop=(i0 == 2 * qc + 1 and 2 * qc + 1 < PBF
                                          and hj0 == 1),
                                    perf_mode=DR,
                                    skip_group_check=True,
                                )
                        else:
                            klo0 = 128 * i0
                            for qc in range(i0 // 4, NQC):
                                lo = max(QCH * qc, klo0)
                                hi = QCH * qc + QCH
                                nc.tensor.matmul(
                                    yps[:, lo:hi], vsp0, at0[:, lo:hi],
                                    start=(i0 == 0 and hj0 == 0),
                                    stop=(i0 == min(TT - 1, 4 * qc + 3) and hj0 == 1),
                                    skip_group_check=True,
                                )

                    def softmax_kt(hj, kt, at_out, j, sums):
                        """S^T + mask + exp for one kt tile; returns rcp."""
                        hl = 2 * hp + hj
                        klo = 128 * kt
                        bc0 = klo // BCH
                        for bc in range(bc0, NBC):
                            blo = max(BCH * bc, klo)
                            s_ps = psS.tile([128, BCH], FP32, tag="s", bufs=2,
                                            name="s_ps")
                            for half in range(2):
                                plo = max(blo, BCH * bc + half * QCH)
                                phi = BCH * bc + (half + 1) * QCH
                                if plo >= phi:
                                    continue
                                nc.tensor.matmul(
                                    s_ps[:, plo - BCH * bc:phi - BCH * bc],
                                    ktp2[:, hp, hj, klo:klo + 128],
                                    qt2[:, hp, plo:phi],
                                    start=True, stop=True,
                                )
                            if bc == bc0:
                                off = klo - BCH * bc
                                nc.vector.tensor_add(
                                    s_ps[:, off:off + 128],
                                    s_ps[:, off:off + 128], mask_sb
                                )
                            dst = (at_out[:, j, blo:BCH * (bc + 1)] if j is not None
                                   else at_out[:, blo:BCH * (bc + 1)])
                            nc.scalar.activation(
                                dst, s_ps[:, blo - BCH * bc:],
                                Exp, scale=SCALE2,
                                accum_out=sums[:, bc:bc + 1],
                            )
                        rcp = sm.tile([128, 1], FP32, tag="rcp", bufs=4, name="rcp")
                        if bc0 == NBC - 1:
                            nc.vector.reciprocal(rcp, sums[:, bc0:NBC])
                        else:
                            stot = sm.tile([128, 1], FP32, tag="stot", bufs=4,
                                           name="stot")
                            nc.vector.reduce_sum(
                                stot, sums[:, bc0:NBC], axis=mybir.AxisListType.X,
                            )
                            nc.vector.reciprocal(rcp, stot)
                        return rcp

                    for p in range(NKP):
                        for hj in range(2):
                            hl = 2 * hp + hj
                            if p < PBF:
                                at2 = atp.tile([128, 2, T], FP8, tag="at", bufs=4,
                                               name="at")
                                # odd kt's never-exp'd 128-col block must read 0
                                nc.gpsimd.memset(
                                    at2[:, 1, 256 * p:256 * p + 128].bitcast(U8), 0
                                )
                                sums = sm.tile([128, 2, NBC], FP32, tag="sums",
                                               bufs=4, name="sums")
                                vsp2 = vspad[hj][p % 2]
                                for j in range(2):
                                    kt = 2 * p + j
                                    rcp = softmax_kt(hj, kt, at2, j, sums[:, j, :])
                                    nc.vector.tensor_scalar(
                                        vsp2[:, j, hj * 64:hj * 64 + 64],
                                        v_sb[:, kt, hl * 64:(hl + 1) * 64], rcp,
                                        WS, mybir.AluOpType.mult,
                                        mybir.AluOpType.mult,
                                    )
                                pend.append(("dr", hj, p, at2, vsp2))
                                if len(pend) > 2:
                                    emit_av(*pend.pop(0))
                            else:
                                for j in range(2):
                                    kt = 2 * p + j
                                    atb = atp.tile([128, T], BF16, tag="atb", bufs=4,
                                                   name="atb")
                                    sums = sm.tile([128, NBC], FP32, tag="sumsb",
                                                   bufs=4, name="sumsb")
                                    rcp = softmax_kt(hj, kt, atb, None, sums)
                                    vspb = vspad_bf[hj][kt % 2]
                                    nc.vector.tensor_scalar(
                                        vspb[:, hj * 64:hj * 64 + 64],
                                        v_sb[:, kt, hl * 64:(hl + 1) * 64], rcp,
                                        WS, mybir.AluOpType.mult,
                                        mybir.AluOpType.mult,
                                    )
                                    pend.append(("bf", hj, kt, atb, vspb))
                                    if len(pend) > 2:
                                        emit_av(*pend.pop(0))
                    for p_ in pend:
                        emit_av(*p_)
                    nc.vector.tensor_copy(yb[:, hp, :], yps)

            # ---- output projection (fp8 DoubleRow pair + single) ----
            with tc.tile_pool(name="psP", bufs=4, space="PSUM") as psP:
                for tt in range(TT):
                    po = psP.tile([128, 768], FP32, tag="ps", bufs=4, name="po")
                    for half, w0, w1 in ((0, 0, QCH), (1, QCH, C)):
                        for et in range(ET):
                            nc.tensor.matmul(
                                po[:, w0:w1],
                                yb[:, et, tt * 128:(tt + 1) * 128],
                                wp_sb[:, et, w0:w1],
                                start=(et == 0), stop=(et == ET - 1),
                                skip_group_check=True,
                            )
                    o_sb = op.tile([128, C], FP32, tag="o", bufs=3, name="o_sb")
                    nc.vector.tensor_scalar_mul(o_sb, po, ODESC)
                    nc.sync.dma_start(out=out[tt * 128:(tt + 1) * 128, :], in_=o_sb)

    _split_sync_waits(nc)
    return nc


_nc_cache = {}
last_result = None


def kernel(x, w_attn, b_attn, w_proj, b_proj):
    global last_result
    if "nc" not in _nc_cache:
        _nc_cache["nc"] = _build()
    nc = _nc_cache["nc"]

    x = np.asarray(x, dtype=np.float32)
    w_attn = np.asarray(w_attn, dtype=np.float32)
    b_attn = np.asarray(b_attn, dtype=np.float32)
    w_proj = np.asarray(w_proj, dtype=np.float32)
    b_proj = np.asarray(b_proj, dtype=np.float32)

    tri = np.where(
        np.arange(128)[None, :] >= np.arange(128)[:, None], 0.0, MASKV
    ).astype(np.float32)

    in_maps = []
    for core in range(NCORES):
        b = core // 2
        e0 = (core % 2) * E
        xt_blk = np.ascontiguousarray(
            x[b].T.reshape(CT, 128, NQC, QCH).transpose(2, 1, 0, 3)
        )
        def _wblk(w, dt):
            return np.ascontiguousarray(
                w.reshape(CT, 128, E).transpose(1, 0, 2)
            ).astype(dt)
        in_maps.append({
            "xT": xt_blk.astype(NPF8),
            "xB": xt_blk.astype(ml_dtypes.bfloat16),
            "wq": _wblk(w_attn[:, e0:e0 + E] * WS, NPF8),
            "wk": _wblk(w_attn[:, C + e0:C + e0 + E] * WS, NPF8),
            "wv": _wblk(w_attn[:, 2 * C + e0:2 * C + e0 + E], ml_dtypes.bfloat16),
            "bq": np.ascontiguousarray(b_attn[e0:e0 + E] * WS),
            "bk": np.ascontiguousarray(b_attn[C + e0:C + e0 + E] * WS),
            "bv": np.ascontiguousarray(b_attn[2 * C + e0:2 * C + e0 + E]),
            "wp": np.ascontiguousarray(
                w_proj[e0:e0 + E, :].reshape(ET, 128, C).transpose(1, 0, 2)
            ).astype(ml_dtypes.bfloat16),
            "mask": tri,
        })

    trace = os.environ.get("ATT_TRACE", "0")
    kw = {}
    if trace != "0":
        n = min(int(trace), NCORES)
        kw = dict(trace=True, trace_cores=list(range(n)))
    res = run_bass_kernel_spmd(nc, in_maps, list(range(NCORES)), **kw)
    last_result = res

    out = np.zeros((B, T, C), dtype=np.float32)
    for core in range(NCORES):
        out[core // 2] += res.results[core]["out"]
    out += b_proj[None, None, :]
    return out
